# revision 18
# baseline (speedup 1.0000x reference)
"""MoE expert-choice routing kernel for 8 TRN2 NeuronCores.

Strategy (expert-parallel, one expert per core):
  host: routing in float64 (logits -> softmax -> top-512 tokens per
        (batch, expert)), gather of selected token rows, operand
        pre-transpose + bf16 pre-cast into DMA-friendly blocked layouts.
  device (per core, expert e): y = silu(xin @ w1[e].T) @ w2[e].T scaled
        by the gates; two chained matmuls with the hidden activations
        kept in SBUF.
  host: scatter-add of the 8 per-expert partial outputs (token indices
        are unique within one (batch, expert) pair).

Perf notes (vs the first working version):
  - All inputs stream in as a handful of large contiguous DMAs split
    across BOTH hardware DGE queues (sync + scalar) instead of ~170
    small ones on a single queue: the ~600ns per-instruction issue cost
    on one queue made the DMA supply issue-limited at startup, stalling
    mm1 until ~26us.
  - Warmup matmuls use a full 512-row moving operand so the HAM power
    manager sees real activity and burns its [full/half]-clock ramp
    cycle during the initial DMA wait, not during real mm1 work.
  - y is returned as bf16 (halves output traffic; ~0.1% extra error).
  - The last output chunk is drained in two halves to shorten the
    post-matmul tail.
"""
import sys

if "/opt/trn_rl_repo" not in sys.path:
    sys.path.insert(0, "/opt/trn_rl_repo")

import numpy as np
import ml_dtypes

B = 4          # batch
S = 2048       # tokens per batch (block size)
D = 1024       # d_model
F = 4096       # d_ffn
E = 8          # experts == cores
K = 512        # tokens per (batch, expert)
T = B * K      # 2048 token rows per core
P = 128
TB = 512       # token block in the device kernel
NB = T // TB   # 4
DT = D // P    # 8
FT = F // P    # 32
FC = 256       # w1 f-chunk width (2 ft columns of 128)
NFC = F // FC  # 16
NWARM = 15

_NC = None
_NAMES = None


def _build():
    """Build + compile the per-core Bass program once."""
    global _NC, _NAMES
    if _NC is not None:
        return _NC, _NAMES

    import concourse.mybir as mybir
    import concourse.tile as tile
    from concourse import bacc

    BF = mybir.dt.bfloat16
    F32 = mybir.dt.float32

    nc = bacc.Bacc(None, target_bir_lowering=False)
    with tile.TileContext(nc) as tc:
        with tc.tile_pool(name="dram", bufs=1, space="DRAM") as dram:
            xb = dram.tile([NB, P, DT * TB], BF, kind="ExternalInput", name="xb")
            w1b = dram.tile([NFC, P, DT * FC], BF, kind="ExternalInput", name="w1b")
            w2b = dram.tile([P, FT * D], BF, kind="ExternalInput", name="w2b")
            g = dram.tile([P, T // P], F32, kind="ExternalInput", name="g")
            y = dram.tile([T, D], BF, kind="ExternalOutput", name="y")

            with (
                tc.tile_pool(name="wpool", bufs=1) as wpool,
                tc.tile_pool(name="xpool", bufs=2) as xpool,
                tc.tile_pool(name="hpool", bufs=1) as hpool,
                tc.tile_pool(name="ps1", bufs=4, space="PSUM") as ps1pool,
                tc.tile_pool(name="ps2", bufs=2, space="PSUM") as ps2pool,
                tc.tile_pool(name="ypool", bufs=4) as ypool,
            ):
                w1s = wpool.tile([P, NFC, DT, FC], BF, name="w1s")
                w2s = wpool.tile([P, FT, D], BF, name="w2s")
                gs = wpool.tile([P, T // P], F32, name="gs")
                warm_w = wpool.tile([P, P], BF, name="warm_w")
                warm_x = wpool.tile([P, TB], BF, name="warm_x")
                nc.vector.memset(warm_w[:], 0)
                nc.vector.memset(warm_x[:], 0)

                # all mm1-gating tensors stream on the sync queue (the
                # scalar queue's transfers start ~2us later); the scalar
                # queue is reserved for y-outs
                xs_t = [None] * NB
                xs_t[0] = xpool.tile([P, DT, TB], BF, name="xs")
                nc.sync.dma_start(xs_t[0][:, 0:4, :], xb[0, :, 0:4 * TB])
                nc.sync.dma_start(w1s[:, 0, :, :], w1b[0])
                nc.sync.dma_start(xs_t[0][:, 4:8, :], xb[0, :, 4 * TB:])
                nc.sync.dma_start(gs[:], g[:])
                for fc in range(1, NFC):
                    nc.sync.dma_start(w1s[:, fc, :, :], w1b[fc])
                xs_t[1] = xpool.tile([P, DT, TB], BF, name="xs")
                nc.sync.dma_start(xs_t[1][:], xb[1])
                nc.sync.dma_start(w2s[:, 0:16, :], w2b[:, 0:16 * D])
                nc.sync.dma_start(w2s[:, 16:32, :], w2b[:, 16 * D:])
                # these two reuse xs buffers -> their issue blocks on mm1 of
                # blocks 0/1 finishing, so they must be last on the queue
                for tb in (2, 3):
                    xs_t[tb] = xpool.tile([P, DT, TB], BF, name="xs")
                    nc.sync.dma_start(xs_t[tb][:], xb[tb])

                # PE warmup: full-width zero matmuls burn the HAM ramp cycle
                # while the first x/w1 DMAs are still in flight.
                ps_warm = ps1pool.tile([P, TB], F32, name="ps1")
                for i in range(NWARM):
                    nc.tensor.matmul(
                        ps_warm[:], warm_w[:], warm_x[:],
                        start=(i == 0), stop=(i == NWARM - 1),
                    )

                for tb in range(NB):
                    xs = xs_t[tb]
                    # mm1: hT[f, t] = silu(w1T.T @ xinT) for this token block
                    hs = hpool.tile([P, FT, TB], BF, name="hs")
                    for ft in range(FT):
                        fc, j = divmod(ft, FC // P)
                        ps = ps1pool.tile([P, TB], F32, name="ps1")
                        for dt in range(DT):
                            nc.tensor.matmul(
                                ps[:],
                                w1s[:, fc, dt, j * P:(j + 1) * P],
                                xs[:, dt, :],
                                start=(dt == 0),
                                stop=(dt == DT - 1),
                            )
                        nc.scalar.activation(
                            hs[:, ft, :], ps[:],
                            mybir.ActivationFunctionType.Silu,
                        )
                    # mm2: y[t, d] = hT.T @ w2T, scaled per-token by gates
                    for tt in range(TB // P):
                        col = tb * (TB // P) + tt
                        ps2 = [
                            ps2pool.tile([P, 512], F32, name=f"ps2_{dc}")
                            for dc in range(D // 512)
                        ]
                        for ft in range(FT):
                            for dc in range(D // 512):
                                nc.tensor.matmul(
                                    ps2[dc][:],
                                    hs[:, ft, tt * P:(tt + 1) * P],
                                    w2s[:, ft, dc * 512:(dc + 1) * 512],
                                    start=(ft == 0),
                                    stop=(ft == FT - 1),
                                )
                        if col < T // P - 1:
                            ys = ypool.tile([P, D], BF, name="ys")
                            for dc in range(D // 512):
                                nc.vector.tensor_scalar_mul(
                                    ys[:, dc * 512:(dc + 1) * 512],
                                    ps2[dc][:], gs[:, col:col + 1],
                                )
                            nc.scalar.dma_start(y[col * P:(col + 1) * P, :], ys[:])
                        else:
                            # final chunk: scale the halves on two engines
                            # (scalar Copy-with-scale + vector) and DMA them
                            # on both queues so the whole drain overlaps
                            ysh0 = ypool.tile([P, 512], BF, name="ysh0")
                            nc.scalar.activation(
                                ysh0[:], ps2[0][:],
                                mybir.ActivationFunctionType.Copy,
                                scale=gs[:, col:col + 1],
                            )
                            nc.scalar.dma_start(
                                y[col * P:(col + 1) * P, 0:512], ysh0[:]
                            )
                            ysh1 = ypool.tile([P, 512], BF, name="ysh1")
                            nc.vector.tensor_scalar_mul(
                                ysh1[:], ps2[1][:], gs[:, col:col + 1]
                            )
                            nc.sync.dma_start(
                                y[col * P:(col + 1) * P, 512:1024], ysh1[:]
                            )
    nc.compile()
    _NC = nc
    _NAMES = (xb.name, w1b.name, w2b.name, g.name, y.name)
    return _NC, _NAMES


def _to_bf16(a):
    """Fast f32 -> bf16 with round-to-nearest-even."""
    a = np.ascontiguousarray(a, dtype=np.float32)
    v = a.view(np.uint32)
    r = ((v + np.uint32(0x7FFF) + ((v >> np.uint32(16)) & np.uint32(1)))
         >> np.uint32(16)).astype(np.uint16)
    return r.view(ml_dtypes.bfloat16)


def _routing(x, choice):
    """float64 routing: per (batch, expert) top-K token ids + gates."""
    logits = np.einsum(
        "bsd,ed->bse",
        x.astype(np.float64), choice.astype(np.float64),
        optimize=True,
    )
    m = logits.max(axis=-1, keepdims=True)
    p = np.exp(logits - m)
    probs = p / p.sum(axis=-1, keepdims=True)  # [b, s, e]
    idx = np.empty((B, E, K), dtype=np.int64)
    gates = np.empty((B, E, K), dtype=np.float32)
    for b in range(B):
        for e in range(E):
            pe = probs[b, :, e]
            ii = np.argpartition(-pe, K)[:K]
            ii = np.sort(ii)
            idx[b, e] = ii
            gates[b, e] = pe[ii].astype(np.float32)
    return idx, gates


def kernel(x, choice, w1, w2):
    from concourse.bass_utils import run_bass_kernel_spmd

    x = np.ascontiguousarray(x, dtype=np.float32)
    choice = np.ascontiguousarray(choice, dtype=np.float32)
    w1 = np.ascontiguousarray(w1, dtype=np.float32)
    w2 = np.ascontiguousarray(w2, dtype=np.float32)
    assert x.shape == (B, S, D) and w1.shape == (E, F, D) and w2.shape == (E, D, F)

    nc, (n_xb, n_w1b, n_w2b, n_g, n_y) = _build()

    idx, gates = _routing(x, choice)

    def _prep(e):
        xin = np.empty((T, D), dtype=np.float32)
        for b in range(B):
            xin[b * K:(b + 1) * K] = x[b, idx[b, e], :]
        # xb[tb, p, dt*TB+j] = xin[tb*TB+j, dt*P+p]
        xbe = np.ascontiguousarray(
            _to_bf16(xin).reshape(NB, TB, DT, P).transpose(0, 3, 2, 1)
        ).reshape(NB, P, DT * TB)
        # w1b[fc, p, dt*FC+jj] = w1[e][fc*FC+jj, dt*P+p]
        w1be = np.ascontiguousarray(
            _to_bf16(w1[e]).reshape(NFC, FC, DT, P).transpose(0, 3, 2, 1)
        ).reshape(NFC, P, DT * FC)
        # w2b[p, ft*D+d] = w2[e].T[ft*P+p, d] = w2[e][d, ft*P+p]
        w2be = np.ascontiguousarray(
            _to_bf16(w2[e]).T.reshape(FT, P, D).transpose(1, 0, 2)
        ).reshape(P, FT * D)
        gflat = gates[:, e].reshape(T)                        # rows b*K + k
        gcols = np.ascontiguousarray(gflat.reshape(T // P, P).T)  # [P, T//P]
        return {n_xb: xbe, n_w1b: w1be, n_w2b: w2be, n_g: gcols}

    from concurrent.futures import ThreadPoolExecutor

    with ThreadPoolExecutor(E) as pool:
        in_maps = list(pool.map(_prep, range(E)))

    res = run_bass_kernel_spmd(nc, in_maps, core_ids=list(range(E)))

    out = np.zeros((B, S, D), dtype=np.float32)
    for e in range(E):
        ye = np.asarray(res.results[e][n_y], dtype=np.float32)  # [T, D]
        for b in range(B):
            out[b, idx[b, e], :] += ye[b * K:(b + 1) * K]
    return out


# revision 19
# speedup vs baseline: 1.1955x; 1.1955x over previous
"""MoE expert-choice routing kernel for 8 TRN2 NeuronCores.

Strategy (expert-parallel, one expert per core):
  host: routing in float64 (logits -> softmax -> top-512 tokens per
        (batch, expert)), gather of selected token rows, operand
        pre-transpose + bf16 pre-cast into DMA-friendly blocked layouts.
  device (per core, expert e): y = silu(xin @ w1[e].T) @ w2[e].T scaled
        by the gates; two chained matmuls with the hidden activations
        kept in SBUF.
  host: scatter-add of the 8 per-expert partial outputs (token indices
        are unique within one (batch, expert) pair).

Perf notes (vs the first working version, 468.6us -> ~461.5us):
  - All inputs stream in as ~22 large contiguous DMAs on the sync
    queue in exact consumption order instead of ~170 small ones: the
    ~600ns per-instruction issue cost made the DMA supply
    issue-limited at startup, stalling mm1 until ~26us.  The scalar
    HWDGE queue (whose transfers start ~2us later) only carries
    y-outs.  The two x blocks that reuse SBUF buffers are last on the
    queue so their WAR waits can't block the FIFO.
  - Warmup matmuls use a full 512-row moving operand sized (NWARM=15)
    so the PE is continuously busy at low p-state from queue start
    until the first x/w1 bytes land (~14us).  This pays the HAM power
    manager's ~6us half-duty dues, after which it grants full clock
    for the entire body; any idle gap here makes the grant a coin
    flip between clean and a [full 3.4us / HALF 6.8us] ramp cycle
    that half-clocks early mm1 work.
  - ps1 has 4 PSUM banks so an mm1 chain never waits on the Silu
    activation that frees its accumulator (2 banks stalled ~200ns
    every ~6 chains).
  - y is returned as bf16 (halves output traffic; ~0.1% extra error).
  - The final output chunk is scaled on two engines (scalar
    Copy-with-scale + vector) and stored via both DMA queues so the
    post-matmul drain fully overlaps.
  Body floor: 2048 matmuls x 215.9ns (512-row bf16 @ 2.4GHz + ~6
  cycles/instr PE overhead) = 441us; fp8 double-pumping is 2x faster
  but e4m3 quantization error (~3.6-5%) fails the 2e-2 gate.
"""
import sys

if "/opt/trn_rl_repo" not in sys.path:
    sys.path.insert(0, "/opt/trn_rl_repo")

import numpy as np
import ml_dtypes

B = 4          # batch
S = 2048       # tokens per batch (block size)
D = 1024       # d_model
F = 4096       # d_ffn
E = 8          # experts == cores
K = 512        # tokens per (batch, expert)
T = B * K      # 2048 token rows per core
P = 128
TB = 512       # token block in the device kernel
NB = T // TB   # 4
DT = D // P    # 8
FT = F // P    # 32
FC = 256       # w1 f-chunk width (2 ft columns of 128)
NFC = F // FC  # 16
NWARM = 15

_NC = None
_NAMES = None


def _build():
    """Build + compile the per-core Bass program once."""
    global _NC, _NAMES
    if _NC is not None:
        return _NC, _NAMES

    import concourse.mybir as mybir
    import concourse.tile as tile
    from concourse import bacc

    BF = mybir.dt.bfloat16
    F32 = mybir.dt.float32

    nc = bacc.Bacc(None, target_bir_lowering=False)
    with tile.TileContext(nc) as tc:
        with tc.tile_pool(name="dram", bufs=1, space="DRAM") as dram:
            xb = dram.tile([NB, P, DT * TB], BF, kind="ExternalInput", name="xb")
            w1b = dram.tile([NFC, P, DT * FC], BF, kind="ExternalInput", name="w1b")
            w2b = dram.tile([P, FT * D], BF, kind="ExternalInput", name="w2b")
            g = dram.tile([P, T // P], F32, kind="ExternalInput", name="g")
            y = dram.tile([T, D], BF, kind="ExternalOutput", name="y")

            with (
                tc.tile_pool(name="wpool", bufs=1) as wpool,
                tc.tile_pool(name="xpool", bufs=2) as xpool,
                tc.tile_pool(name="hpool", bufs=1) as hpool,
                tc.tile_pool(name="ps1", bufs=4, space="PSUM") as ps1pool,
                tc.tile_pool(name="ps2", bufs=2, space="PSUM") as ps2pool,
                tc.tile_pool(name="ypool", bufs=4) as ypool,
            ):
                w1s = wpool.tile([P, NFC, DT, FC], BF, name="w1s")
                w2s = wpool.tile([P, FT, D], BF, name="w2s")
                gs = wpool.tile([P, T // P], F32, name="gs")
                warm_w = wpool.tile([P, P], BF, name="warm_w")
                warm_x = wpool.tile([P, TB], BF, name="warm_x")
                nc.vector.memset(warm_w[:], 0)
                nc.vector.memset(warm_x[:], 0)

                # all mm1-gating tensors stream on the sync queue (the
                # scalar queue's transfers start ~2us later); the scalar
                # queue is reserved for y-outs
                xs_t = [None] * NB
                xs_t[0] = xpool.tile([P, DT, TB], BF, name="xs")
                nc.sync.dma_start(xs_t[0][:, 0:4, :], xb[0, :, 0:4 * TB])
                nc.sync.dma_start(w1s[:, 0, :, :], w1b[0])
                nc.sync.dma_start(xs_t[0][:, 4:8, :], xb[0, :, 4 * TB:])
                nc.sync.dma_start(gs[:], g[:])
                for fc in range(1, NFC):
                    nc.sync.dma_start(w1s[:, fc, :, :], w1b[fc])
                xs_t[1] = xpool.tile([P, DT, TB], BF, name="xs")
                nc.sync.dma_start(xs_t[1][:], xb[1])
                nc.sync.dma_start(w2s[:, 0:16, :], w2b[:, 0:16 * D])
                nc.sync.dma_start(w2s[:, 16:32, :], w2b[:, 16 * D:])
                # these two reuse xs buffers -> their issue blocks on mm1 of
                # blocks 0/1 finishing, so they must be last on the queue
                for tb in (2, 3):
                    xs_t[tb] = xpool.tile([P, DT, TB], BF, name="xs")
                    nc.sync.dma_start(xs_t[tb][:], xb[tb])

                # PE warmup: full-width zero matmuls burn the HAM ramp cycle
                # while the first x/w1 DMAs are still in flight.
                ps_warm = ps1pool.tile([P, TB], F32, name="ps1")
                for i in range(NWARM):
                    nc.tensor.matmul(
                        ps_warm[:], warm_w[:], warm_x[:],
                        start=(i == 0), stop=(i == NWARM - 1),
                    )

                for tb in range(NB):
                    xs = xs_t[tb]
                    # mm1: hT[f, t] = silu(w1T.T @ xinT) for this token block
                    hs = hpool.tile([P, FT, TB], BF, name="hs")
                    for ft in range(FT):
                        fc, j = divmod(ft, FC // P)
                        ps = ps1pool.tile([P, TB], F32, name="ps1")
                        for dt in range(DT):
                            nc.tensor.matmul(
                                ps[:],
                                w1s[:, fc, dt, j * P:(j + 1) * P],
                                xs[:, dt, :],
                                start=(dt == 0),
                                stop=(dt == DT - 1),
                            )
                        nc.scalar.activation(
                            hs[:, ft, :], ps[:],
                            mybir.ActivationFunctionType.Silu,
                        )
                    # mm2: y[t, d] = hT.T @ w2T, scaled per-token by gates
                    for tt in range(TB // P):
                        col = tb * (TB // P) + tt
                        ps2 = [
                            ps2pool.tile([P, 512], F32, name=f"ps2_{dc}")
                            for dc in range(D // 512)
                        ]
                        for ft in range(FT):
                            for dc in range(D // 512):
                                nc.tensor.matmul(
                                    ps2[dc][:],
                                    hs[:, ft, tt * P:(tt + 1) * P],
                                    w2s[:, ft, dc * 512:(dc + 1) * 512],
                                    start=(ft == 0),
                                    stop=(ft == FT - 1),
                                )
                        if col < T // P - 1:
                            ys = ypool.tile([P, D], BF, name="ys")
                            for dc in range(D // 512):
                                nc.vector.tensor_scalar_mul(
                                    ys[:, dc * 512:(dc + 1) * 512],
                                    ps2[dc][:], gs[:, col:col + 1],
                                )
                            nc.scalar.dma_start(y[col * P:(col + 1) * P, :], ys[:])
                        else:
                            # final chunk: scale the halves on two engines
                            # (scalar Copy-with-scale + vector) and DMA them
                            # on both queues so the whole drain overlaps
                            ysh0 = ypool.tile([P, 512], BF, name="ysh0")
                            nc.scalar.activation(
                                ysh0[:], ps2[0][:],
                                mybir.ActivationFunctionType.Copy,
                                scale=gs[:, col:col + 1],
                            )
                            nc.scalar.dma_start(
                                y[col * P:(col + 1) * P, 0:512], ysh0[:]
                            )
                            ysh1 = ypool.tile([P, 512], BF, name="ysh1")
                            nc.vector.tensor_scalar_mul(
                                ysh1[:], ps2[1][:], gs[:, col:col + 1]
                            )
                            nc.sync.dma_start(
                                y[col * P:(col + 1) * P, 512:1024], ysh1[:]
                            )
    nc.compile()
    _NC = nc
    _NAMES = (xb.name, w1b.name, w2b.name, g.name, y.name)
    return _NC, _NAMES


def _to_bf16(a):
    """Fast f32 -> bf16 with round-to-nearest-even."""
    a = np.ascontiguousarray(a, dtype=np.float32)
    v = a.view(np.uint32)
    r = ((v + np.uint32(0x7FFF) + ((v >> np.uint32(16)) & np.uint32(1)))
         >> np.uint32(16)).astype(np.uint16)
    return r.view(ml_dtypes.bfloat16)


def _routing(x, choice):
    """float64 routing: per (batch, expert) top-K token ids + gates."""
    logits = np.einsum(
        "bsd,ed->bse",
        x.astype(np.float64), choice.astype(np.float64),
        optimize=True,
    )
    m = logits.max(axis=-1, keepdims=True)
    p = np.exp(logits - m)
    probs = p / p.sum(axis=-1, keepdims=True)  # [b, s, e]
    idx = np.empty((B, E, K), dtype=np.int64)
    gates = np.empty((B, E, K), dtype=np.float32)
    for b in range(B):
        for e in range(E):
            pe = probs[b, :, e]
            ii = np.argpartition(-pe, K)[:K]
            ii = np.sort(ii)
            idx[b, e] = ii
            gates[b, e] = pe[ii].astype(np.float32)
    return idx, gates


def kernel(x, choice, w1, w2):
    from concourse.bass_utils import run_bass_kernel_spmd

    x = np.ascontiguousarray(x, dtype=np.float32)
    choice = np.ascontiguousarray(choice, dtype=np.float32)
    w1 = np.ascontiguousarray(w1, dtype=np.float32)
    w2 = np.ascontiguousarray(w2, dtype=np.float32)
    assert x.shape == (B, S, D) and w1.shape == (E, F, D) and w2.shape == (E, D, F)

    nc, (n_xb, n_w1b, n_w2b, n_g, n_y) = _build()

    idx, gates = _routing(x, choice)

    def _prep(e):
        xin = np.empty((T, D), dtype=np.float32)
        for b in range(B):
            xin[b * K:(b + 1) * K] = x[b, idx[b, e], :]
        # xb[tb, p, dt*TB+j] = xin[tb*TB+j, dt*P+p]
        xbe = np.ascontiguousarray(
            _to_bf16(xin).reshape(NB, TB, DT, P).transpose(0, 3, 2, 1)
        ).reshape(NB, P, DT * TB)
        # w1b[fc, p, dt*FC+jj] = w1[e][fc*FC+jj, dt*P+p]
        w1be = np.ascontiguousarray(
            _to_bf16(w1[e]).reshape(NFC, FC, DT, P).transpose(0, 3, 2, 1)
        ).reshape(NFC, P, DT * FC)
        # w2b[p, ft*D+d] = w2[e].T[ft*P+p, d] = w2[e][d, ft*P+p]
        w2be = np.ascontiguousarray(
            _to_bf16(w2[e]).T.reshape(FT, P, D).transpose(1, 0, 2)
        ).reshape(P, FT * D)
        gflat = gates[:, e].reshape(T)                        # rows b*K + k
        gcols = np.ascontiguousarray(gflat.reshape(T // P, P).T)  # [P, T//P]
        return {n_xb: xbe, n_w1b: w1be, n_w2b: w2be, n_g: gcols}

    from concurrent.futures import ThreadPoolExecutor

    with ThreadPoolExecutor(E) as pool:
        in_maps = list(pool.map(_prep, range(E)))

    res = run_bass_kernel_spmd(nc, in_maps, core_ids=list(range(E)))

    out = np.zeros((B, S, D), dtype=np.float32)
    for e in range(E):
        ye = np.asarray(res.results[e][n_y], dtype=np.float32)  # [T, D]
        for b in range(B):
            out[b, idx[b, e], :] += ye[b * K:(b + 1) * K]
    return out
